# revision 4
# baseline (speedup 1.0000x reference)
"""ChebyKAN layer kernel for 8x TRN2 NeuronCores.

y = silu(einsum('bid,iod->bo', stack([T0..T4](tanh(LN(x)))), C))

Strategy: data-parallel over batch B=4096 -> 8 cores x 512 rows.
Per core:
  - LN stats + normalize in [b, i] layout (bn_stats/bn_aggr, ACT affine)
  - PE-transpose 128x128 blocks of z; fuse tanh(z.T*w+b) into the
    PSUM->SBUF evacuation on ScalarE -> hT in [i, b] layout
  - Chebyshev recurrence T2..T4 on DVE in fp32, cast to bf16
  - y.T[o, b] = sum_{d=1..4, i} C_d[i_tile, o_tile].T @ T_d[i_tile, b]
    accumulated over 32 k-tiles in PSUM (bf16 matmuls, fp32 accum)
  - T0 == 1 contributes beta[o] = sum_i C0[i, o]: computed on device by
    8 ones-vector matmuls per o-tile, then fused as the bias of the
    silu() PSUM->SBUF evacuation.
Host: relayout C into [o_tile, p, kslot, m] bf16, transpose y.T shards back.
"""

import numpy as np
import ml_dtypes

import concourse.bacc as bacc
import concourse.tile as tile
import concourse.mybir as mybir
from concourse import bass_utils
from contextlib import ExitStack

# Problem constants (hardcoded per contract).
B, I, O, DEG = 4096, 1024, 1024, 4
NCORES = 8
BS = B // NCORES          # 512 rows per core
P = 128
IT, OT = I // P, O // P   # 8, 8
NCHUNK = 2                # b-chunks per core (pipeline phase1 under matmuls)
BC = BS // NCHUNK         # 256
KMAIN = IT * DEG          # 32 main k-tiles (d=1..4)
KW = IT + KMAIN           # 40 slots per o-tile (8 C0 tiles + 32 main)
LN_EPS = 1e-5

F32 = mybir.dt.float32
BF16 = mybir.dt.bfloat16
AF = mybir.ActivationFunctionType
ALU = mybir.AluOpType


def build_nc():
    nc = bacc.Bacc(
        "TRN2", target_bir_lowering=False, debug=False, enable_asserts=False
    )
    x_d = nc.dram_tensor("x", [BS, I], F32, kind="ExternalInput")
    w_d = nc.dram_tensor("w", [OT, P, KW, P], BF16, kind="ExternalInput")
    lnwb_d = nc.dram_tensor("lnwb", [P, 2 * IT], F32, kind="ExternalInput")
    id_d = nc.dram_tensor("ident", [P, P], F32, kind="ExternalInput")
    y_d = nc.dram_tensor("y_t", [O, BS], F32, kind="ExternalOutput")

    with tile.TileContext(nc) as tc, ExitStack() as ctx:
        const_pool = ctx.enter_context(tc.tile_pool(name="const", bufs=1))
        w_pool = ctx.enter_context(tc.tile_pool(name="wp", bufs=OT))
        x_pool = ctx.enter_context(tc.tile_pool(name="xp", bufs=2))
        stat_pool = ctx.enter_context(tc.tile_pool(name="stp", bufs=3))
        z_pool = ctx.enter_context(tc.tile_pool(name="zp", bufs=2))
        ht_pool = ctx.enter_context(tc.tile_pool(name="htp", bufs=2 * IT))
        tmp_pool = ctx.enter_context(tc.tile_pool(name="tmp", bufs=2))
        s_pool = ctx.enter_context(tc.tile_pool(name="sp", bufs=2 * KMAIN))
        beta_pool = ctx.enter_context(tc.tile_pool(name="bet", bufs=OT))
        out_pool = ctx.enter_context(tc.tile_pool(name="op", bufs=4))
        ps_tr = ctx.enter_context(tc.tile_pool(name="pstr", bufs=2, space="PSUM"))
        ps_beta = ctx.enter_context(tc.tile_pool(name="psb", bufs=2, space="PSUM"))
        ps_main = ctx.enter_context(tc.tile_pool(name="psm", bufs=3, space="PSUM"))

        # --- constants ---
        ident = const_pool.tile([P, P], F32, name="ident", tag="ident")
        nc.sync.dma_start(ident[:], id_d.ap())
        lnwb = const_pool.tile([P, 2 * IT], F32, name="lnwb", tag="lnwb")
        nc.sync.dma_start(lnwb[:], lnwb_d.ap())
        ones_b = const_pool.tile([P, 1], BF16, name="ones_b", tag="ones_b")
        nc.any.memset(ones_b[:], 1.0)
        eps_t = const_pool.tile([P, 1], F32, name="eps_t", tag="eps_t")
        nc.any.memset(eps_t[:], LN_EPS)

        # --- weight macro-tiles: one [128, KW*128] bf16 tile per o-tile ---
        w_tiles = []
        for o in range(OT):
            wt = w_pool.tile([P, KW * P], BF16, name=f"w{o}", tag="w")
            nc.sync.dma_start(wt[:], w_d.ap()[o].rearrange("p k m -> p (k m)"))
            w_tiles.append(wt)

        # --- beta[o] = sum_i C0[i, o] via ones-vector matmuls (also PE warmup)
        beta_sb = []
        for o in range(OT):
            bp = ps_beta.tile([P, 1], F32, name=f"bp{o}", tag="bp")
            for kk in range(IT):
                nc.tensor.matmul(
                    bp[:],
                    w_tiles[o][:, kk * P : (kk + 1) * P],
                    ones_b[:],
                    start=(kk == 0),
                    stop=(kk == IT - 1),
                )
            bsb = beta_pool.tile([P, 1], F32, name=f"beta{o}", tag="beta")
            nc.scalar.copy(bsb[:], bp[:])
            beta_sb.append(bsb)

        ht_tiles = {}
        s_tiles = {}
        for c in range(NCHUNK):
            # --- phase 1: LN + tanh + transpose for this chunk ---
            for tt in range(BC // P):
                t = c * (BC // P) + tt
                xt = x_pool.tile([P, I], F32, name=f"x{t}", tag="x")
                nc.sync.dma_start(xt[:], x_d.ap()[t * P : (t + 1) * P, :])
                stats = stat_pool.tile([P, 12], F32, name=f"st{t}", tag="st")
                nc.vector.bn_stats(stats[:, 0:6], xt[:, 0:512])
                nc.vector.bn_stats(stats[:, 6:12], xt[:, 512:1024])
                mv = stat_pool.tile([P, 2], F32, name=f"mv{t}", tag="mv")
                nc.vector.bn_aggr(mv[:], stats[:])
                std = stat_pool.tile([P, 1], F32, name=f"sd{t}", tag="sd")
                nc.scalar.activation(std[:], mv[:, 1:2], AF.Sqrt, bias=eps_t[:])
                rs = stat_pool.tile([P, 1], F32, name=f"rs{t}", tag="rs")
                nc.vector.reciprocal(rs[:], std[:])
                nb = stat_pool.tile([P, 1], F32, name=f"nb{t}", tag="nb")
                nc.vector.scalar_tensor_tensor(
                    nb[:], mv[:, 0:1], -1.0, rs[:], ALU.mult, ALU.mult
                )
                # z = x * rs + nb   (per-partition scale/bias on ACT)
                zt = z_pool.tile([P, I], F32, name=f"z{t}", tag="z")
                nc.scalar.activation(zt[:], xt[:], AF.Identity, bias=nb[:], scale=rs[:])
                for it in range(IT):
                    if tt == 0:
                        ht_tiles[(c, it)] = ht_pool.tile(
                            [P, BC], F32, name=f"h{c}_{it}", tag="ht"
                        )
                    ht = ht_tiles[(c, it)]
                    ps = ps_tr.tile([P, P], F32, name=f"tr{t}_{it}", tag="tr")
                    nc.tensor.transpose(ps[:], zt[:, it * P : (it + 1) * P], ident[:])
                    # hT = tanh(z.T * ln_w + ln_b), PSUM -> SBUF on ScalarE
                    nc.scalar.activation(
                        ht[:, tt * P : (tt + 1) * P],
                        ps[:],
                        AF.Tanh,
                        bias=lnwb[:, IT + it : IT + it + 1],
                        scale=lnwb[:, it : it + 1],
                    )

            # --- Chebyshev recurrence + bf16 casts, per i-tile ---
            for it in range(IT):
                ht = ht_tiles[(c, it)]
                s1 = s_pool.tile([P, BC], BF16, name=f"s{c}_{it}_0", tag="s")
                nc.gpsimd.tensor_copy(s1[:], ht[:])
                hh = tmp_pool.tile([P, BC], F32, name=f"hh{c}_{it}", tag="hh")
                nc.scalar.activation(hh[:], ht[:], AF.Square)
                t2 = tmp_pool.tile([P, BC], F32, name=f"t2{c}_{it}", tag="t2")
                nc.vector.tensor_scalar(t2[:], hh[:], 2.0, -1.0, ALU.mult, ALU.add)
                s2 = s_pool.tile([P, BC], BF16, name=f"s{c}_{it}_1", tag="s")
                nc.gpsimd.tensor_copy(s2[:], t2[:])
                g3 = tmp_pool.tile([P, BC], F32, name=f"g3{c}_{it}", tag="g3")
                nc.vector.tensor_tensor(g3[:], ht[:], t2[:], ALU.mult)
                t3 = tmp_pool.tile([P, BC], F32, name=f"t3{c}_{it}", tag="t3")
                nc.vector.scalar_tensor_tensor(
                    t3[:], g3[:], 2.0, ht[:], ALU.mult, ALU.subtract
                )
                s3 = s_pool.tile([P, BC], BF16, name=f"s{c}_{it}_2", tag="s")
                nc.gpsimd.tensor_copy(s3[:], t3[:])
                g4 = tmp_pool.tile([P, BC], F32, name=f"g4{c}_{it}", tag="g4")
                nc.vector.tensor_tensor(g4[:], ht[:], t3[:], ALU.mult)
                s4 = s_pool.tile([P, BC], BF16, name=f"s{c}_{it}_3", tag="s")
                nc.vector.scalar_tensor_tensor(
                    s4[:], g4[:], 2.0, t2[:], ALU.mult, ALU.subtract
                )
                for d, s in enumerate((s1, s2, s3, s4)):
                    s_tiles[(c, it, d)] = s

            # --- main matmuls: y.T[o_tile, chunk] += W_kk.T @ S_kk ---
            for o in range(OT):
                pm = ps_main.tile([P, BC], F32, name=f"pm{c}_{o}", tag="pm")
                for it in range(IT):
                    for d in range(DEG):
                        kk = IT + it * DEG + d
                        nc.tensor.matmul(
                            pm[:],
                            w_tiles[o][:, kk * P : (kk + 1) * P],
                            s_tiles[(c, it, d)][:],
                            start=(it == 0 and d == 0),
                            stop=(it == IT - 1 and d == DEG - 1),
                        )
                ot_ = out_pool.tile([P, BC], F32, name=f"o{c}_{o}", tag="o")
                # silu(y + beta) fused into the PSUM evacuation
                nc.scalar.activation(ot_[:], pm[:], AF.Silu, bias=beta_sb[o][:])
                nc.sync.dma_start(
                    y_d.ap()[o * P : (o + 1) * P, c * BC : (c + 1) * BC], ot_[:]
                )

    nc.compile()
    return nc


def _prep_inputs(x, cheby_coeffs, ln_weight, ln_bias):
    """Host-side relayout/sharding. Returns in_maps for the 8 cores."""
    x = np.asarray(x, dtype=np.float32)
    C = np.asarray(cheby_coeffs, dtype=np.float32)
    lw = np.asarray(ln_weight, dtype=np.float32)
    lb = np.asarray(ln_bias, dtype=np.float32)

    Cr = C.reshape(IT, P, OT, P, DEG + 1)
    main = Cr[:, :, :, :, 1:].transpose(2, 1, 0, 4, 3).reshape(OT, P, KMAIN, P)
    c0 = Cr[:, :, :, :, 0].transpose(2, 1, 0, 3)  # [OT, P, IT, P]
    w = np.ascontiguousarray(
        np.concatenate([c0, main], axis=2), dtype=np.float32
    ).astype(ml_dtypes.bfloat16)

    lnwb = np.concatenate(
        [lw.reshape(IT, P).T, lb.reshape(IT, P).T], axis=1
    ).astype(np.float32)
    lnwb = np.ascontiguousarray(lnwb)
    ident = np.eye(P, dtype=np.float32)

    in_maps = []
    for c in range(NCORES):
        in_maps.append(
            {
                "x": np.ascontiguousarray(x[c * BS : (c + 1) * BS]),
                "w": w,
                "lnwb": lnwb,
                "ident": ident,
            }
        )
    return in_maps


_NC_CACHE = None


def _get_nc():
    global _NC_CACHE
    if _NC_CACHE is None:
        _NC_CACHE = build_nc()
    return _NC_CACHE


def kernel(x, cheby_coeffs, ln_weight, ln_bias):
    nc = _get_nc()
    in_maps = _prep_inputs(x, cheby_coeffs, ln_weight, ln_bias)
    res = bass_utils.run_bass_kernel_spmd(
        nc, in_maps, core_ids=list(range(NCORES)), trace=False
    )
    y = np.empty((B, O), dtype=np.float32)
    for c in range(NCORES):
        y[c * BS : (c + 1) * BS, :] = res.results[c]["y_t"].T
    return y
